# revision 23
# baseline (speedup 1.0000x reference)
"""Trainium2 Bass kernel for AttnApply (sliding-window weighted sum).

out[b, t, c] = sum_i padded[b, t+i, c] * weights[b, t, i]   (T=11, D=5 zero pad)

Strategy
--------
Pure data parallel over batch: 8 cores x 4 batches each.

Per core, the windowed sum is a banded matrix multiply on the TensorEngine:
for a group of M=32 output rows starting at t0 (window = 42 input rows),

    psum[c, m] = sum_k in_pad[t0+k, c] * band[k, m]

with band[k, m] = w[t0+m, k-m] for 0 <= k-m < T.  PSUM partitions are the
channel half (2 x 128), free dim is time; stores are channel-major [C, L]
(host un-transposes).

v4 layout (vs the M=118 blocked v1): the input ships UNBLOCKED — rows
time-major in 128-row chunks ([128, NCH, C] partition-major DRAM, one long
contiguous run per partition) with no per-block overlap materialization
(-8% input bytes).  Group i (phase f = i%4, chunk q = i//4) has its window
at chunk-q partitions [32f, 32f+42).

HW constraint (measured, mmtest6): matmuls at DIFFERENT PE tile row
positions in one program fault intermittently (LDWEIGHTS/matmul reorder
race), so EVERY matmul sits at base partition 0 (tile position (0,0);
mixed tile SIZES at base 0 measured reliable, mmtest7 12/12).  Phase f
contracts chunk rows [0, 32f+42); the band region's rows [0, 32f) are
structurally ZERO and live permanently in SBUF (memset once), while band
DMAs only ever write each region's 42 value rows ([32f, 32f+42); wrap
region: [0,10)).  ONE K=128 matmul per (chunk, half) covers all 4 phases:
its moving operand spans the 4 regions at that chunk's columns ([128, 4,
32] AP, free size 128) and the zero rows make each phase's psum columns
pick up only its own window; a second matmul accumulates the wrap piece
(phase 3, next chunk's first 10 rows) — ALSO at K=128 (region-4 rows
[10,128) are persistent zeros) because mixing PE tile sizes measured
~150ns of stall per 128<->32 transition (~20us/core).  Every matmul in
the program is an identical (128,128)@(0,0) tile.

Shipped band: 168 rows x 64B per 4-group set = 84B per output row vs 256B
for the v1 dense K=128 band (-67%).  Total HBM traffic 4.61MB/batch vs
5.45 for v1 (-15%).

DMA-count discipline (v3 post-mortem: each dma_start costs ~0.6us of
serialized HWDGE/SWDGE descriptor generation, which burned the byte
savings): per batch ONE input DMA (the 60KB zero tail rows ship rather
than paying a second descriptor round), TWO stores (one per channel half,
on the ACT-HWDGE and SWDGE queues — splitting the write stream across
queues measured ~7us faster than one merged store DMA), and the 5 band
region DMAs carry TWO batches each (cols interleave batch pairs) =
2.5/batch.  Queue balance per batch: SP-HWDGE input 2.16MB, ACT-HWDGE
store 1.05MB, SWDGE band+store 1.39MB.

Measured (burst-slope method, 8 cores): 70.0us (v1 baseline) -> ~54-60us
(best official test.py run: 53.7us, rel err 2.775e-3).

Everything runs in bf16 (gate is rel_err < 2e-2, measured ~3e-3): bf16
inputs/band, fp32 PSUM accumulation, bf16 stores (host widens to f32).
"""

import ml_dtypes
import numpy as np

import concourse.bass as bass  # noqa: F401  (engine handles hang off nc)
import concourse.mybir as mybir
import concourse.tile as tile
from concourse import bacc
from concourse.bass_utils import run_bass_kernel_spmd

B, L, C, T = 32, 4096, 256, 11
D = T // 2
N_CORES = 8
B_LOC = B // N_CORES            # 4 batches per core
NPAIR = B_LOC // 2              # band DMAs carry batch pairs
NCH = 33                        # 128-row input chunks (32 full + 10-row tail)
LPAD = NCH * 128                # 4224 padded input rows (used: 4106)
GCH = L // 128                  # 32 output groups per chunk-column (g')

_CACHE: dict = {}
LAST_RESULT = None  # BassKernelResults of the most recent run (for test.py)

# engine assignment knobs: queues for input/band/store DMAs and engines for
# the psum->SBUF compact copies
CFG = {
    "group_m": 32,              # output rows per matmul group (32 or 64)
    "input_q": "sync",          # input loads (SP HWDGE)
    "band_q": "gpsimd",         # band loads (SWDGE); may be a list, cycled
    "store_q": ["scalar", "gpsimd"],  # per-half store queues (see split)
    "store_split": True,        # True: one DMA per channel half (queues
                                # cycled), False: single [128,2,L] DMA
    "store_chunks": 1,          # col-chunks per half-store (2 = issue the
                                # first half's write as soon as its copies
                                # land, overlapping the batch tail)
    "copy0": "vector",          # ch-half-0 psum compact copy (DVE)
    "copy1": "scalar",          # ch-half-1 psum compact copy (ACT)
    "in_bufs": 2,
    "in_pair": False,           # one input DMA per batch pair (4.3MB)
    "in_split": None,           # optional [(queue, n_chunks), ...] to split
                                # the input DMA across DGE paths, balancing
                                # per-queue bytes (e.g. 23/7/3 sync/scalar/
                                # gpsimd equalizes all paths at 1.53MB/batch)
    "o_bufs": 3,
    "bd_bufs": 2,               # band ring (each buffer holds 2 batches)
    "band_prefetch": False,     # issue ALL band DMAs at the top of each
                                # repeat so they are not stuck in SWDGE FIFO
                                # behind stores that wait on compute
    # bench-only ablation knobs (produce WRONG results; timing diagnostics)
    "abl_no_pair": False,       # skip the K=10 wrap matmuls
    "abl_no_main": False,       # skip the K=128 main matmuls
    "abl_no_band": False,       # skip band DMAs
    "abl_no_input": False,      # skip input DMAs
    "abl_no_store": False,      # skip store DMAs
    "abl_no_copy": False,       # skip psum->SBUF copies
}

NREG = {32: 5, 64: 3}
# per-region shipped row spans (the value rows; everything else is
# persistent zeros)
SPANS = {
    32: {0: (0, 42), 1: (32, 74), 2: (64, 106), 3: (96, 128), 4: (0, 10)},
    64: {0: (0, 74), 1: (64, 128), 2: (0, 10)},
}


def _eng(nc, name):
    return {
        "sync": nc.sync,
        "scalar": nc.scalar,
        "vector": nc.vector,
        "gpsimd": nc.gpsimd,
        "tensor": nc.tensor,
    }[name]


def _englist(nc, names):
    names = [names] if isinstance(names, str) else names
    return [_eng(nc, n) for n in names]


def _build_nc(repeat: int = 1, bench: bool = False, cfg: dict | None = None):
    """Build the bass program. `repeat` re-runs the whole body N times and
    `bench=True` uses internal zero-filled DRAM inputs/outputs with only a
    tiny external "tick" output — both used only for benchmarking; the
    grading path uses repeat=1, bench=False."""
    cfg = {**CFG, **(cfg or {})}
    M = cfg["group_m"]
    NPH = 128 // M              # phases (4 for M=32, 2 for M=64)
    REGW = GCH * M              # band region width per batch (1024/2048)
    nreg = NREG[M]
    spans = SPANS[M]
    nc = bacc.Bacc(
        "TRN2",
        target_bir_lowering=False,
        debug=False,
        num_devices=N_CORES,
    )
    # partition-major DRAM layouts: SBUF partition dim first so each
    # partition's DMA read is one long contiguous run
    kind_in = {} if bench else {"kind": "ExternalInput"}
    sfx = "_int" if bench else ""
    inp = nc.dram_tensor(
        f"in_chunks{sfx}", [B_LOC, 128, NCH, C], mybir.dt.bfloat16, **kind_in
    ).ap()
    band = nc.dram_tensor(
        f"band{sfx}",
        [NPAIR, 128, nreg, 2, REGW],
        mybir.dt.bfloat16,
        **kind_in,
    ).ap()
    outT = nc.dram_tensor(
        f"outT{sfx}",
        [B_LOC, C, L],
        mybir.dt.bfloat16,
        **({} if bench else {"kind": "ExternalOutput"}),
    ).ap()
    if bench:
        tick = nc.dram_tensor(
            "tick", [1, C], mybir.dt.bfloat16, kind="ExternalOutput"
        ).ap()
    else:
        tick = None

    with tile.TileContext(nc) as tc:
        with (
            tc.tile_pool(name="inp", bufs=cfg["in_bufs"]) as in_pool,
            tc.tile_pool(name="bnd", bufs=1) as bd_pool,
            tc.tile_pool(name="outp", bufs=cfg["o_bufs"]) as o_pool,
            tc.tile_pool(name="ps", bufs=4, space="PSUM") as ps_pool,
        ):
            if bench:
                # back every DRAM page with zeros once per run so reads are
                # real HBM traffic (unbacked-page reads measure absurdly
                # fast and would not represent the grading path)
                with tc.tile_pool(name="z", bufs=1) as z_pool:
                    z = z_pool.tile([128, 2048], mybir.dt.float32, tag="z")
                    nc.gpsimd.memset(z[:, :], 0.0)
                    zb = z.bitcast(mybir.dt.bfloat16)  # [128, 4096] bf16
                    zw = 4096
                    for b in range(B_LOC):
                        flat_in = inp[b].rearrange("p j c -> p (j c)")
                        for r0 in range(0, NCH * C, zw):
                            cnt = min(zw, NCH * C - r0)
                            nc.sync.dma_start(
                                out=flat_in[:, r0 : r0 + cnt],
                                in_=zb[:, :cnt],
                            )
                        for ch in range(2):
                            nc.sync.dma_start(
                                out=outT[b, ch * 128 : (ch + 1) * 128, :],
                                in_=zb[:, :L],
                            )
                    for pr in range(NPAIR):
                        flat_bd = band[pr].rearrange("p f e w -> p (f e w)")
                        for r0 in range(0, nreg * 2 * REGW, zw):
                            cnt = min(zw, nreg * 2 * REGW - r0)
                            nc.sync.dma_start(
                                out=flat_bd[:, r0 : r0 + cnt],
                                in_=zb[:, :cnt],
                            )

            q_in = _englist(nc, cfg["input_q"])
            q_bds = _englist(nc, cfg["band_q"])
            q_sts = _englist(nc, cfg["store_q"])
            e_cp = [_eng(nc, cfg["copy0"]), _eng(nc, cfg["copy1"])]

            def _copy(eng, dst, src):
                if eng is nc.scalar:
                    eng.copy(out=dst, in_=src)
                else:
                    eng.tensor_copy(out=dst, in_=src)

            n_groups = L // M           # groups per batch (128 / 64)
            g_per_ps = 512 // M         # groups per psum tile (one 2KB bank)

            # persistent double-buffered band tiles (2 batches each): zero
            # rows are established ONCE here and survive all batches and
            # repeats — band DMAs only overwrite the value rows
            bd_ts = []
            for i in range(cfg["bd_bufs"]):
                bt = bd_pool.tile(
                    [128, nreg * 2 * REGW],
                    mybir.dt.bfloat16,
                    tag=f"bd{i}",
                    name=f"bd{i}",
                )
                nc.vector.memset(bt[:, :], 0.0)
                bd_ts.append(bt)

            def _band_load(pr):
                bd_t = bd_ts[pr % cfg["bd_bufs"]]
                for rg in range(nreg if not cfg["abl_no_band"] else 0):
                    lo, hi = spans[rg]
                    q_bds[rg % len(q_bds)].dma_start(
                        out=bd_t[
                            lo:hi,
                            rg * 2 * REGW : (rg + 1) * 2 * REGW,
                        ],
                        in_=band[pr, lo:hi, rg].rearrange(
                            "p e w -> p (e w)"
                        ),
                    )
                return bd_t

            for _rep in range(repeat):
                if cfg["band_prefetch"]:
                    for pr in range(NPAIR):
                        _band_load(pr)
                for b in range(B_LOC):
                    pr, e = b // 2, b % 2
                    # ---- band pair load: one DMA per region, 2 batches ----
                    if e == 0:
                        if cfg["band_prefetch"]:
                            bd_t = bd_ts[pr % cfg["bd_bufs"]]
                        else:
                            bd_t = _band_load(pr)
                        bd_r = bd_t.rearrange(
                            "p (r e w) -> p r e w", r=nreg, e=2
                        )
                    # ---- input load: ONE DMA (incl. zero tail rows) ----
                    if cfg["in_pair"]:
                        if e == 0:
                            in_b2 = in_pool.tile(
                                [128, 2 * NCH * C], mybir.dt.bfloat16,
                                tag="in",
                            )
                            if not cfg["abl_no_input"]:
                                q_in[pr % len(q_in)].dma_start(
                                    out=in_b2.rearrange(
                                        "p (b w) -> p b w", b=2
                                    ),
                                    in_=inp[2 * pr : 2 * pr + 2].rearrange(
                                        "b p j c -> p b (j c)"
                                    ),
                                )
                        in_b = in_b2[:, e * NCH * C : (e + 1) * NCH * C]
                    else:
                        in_b = in_pool.tile(
                            [128, NCH * C], mybir.dt.bfloat16, tag="in"
                        )
                        if not cfg["abl_no_input"]:
                            if cfg["in_split"]:
                                c0 = 0
                                for qn, nchk in cfg["in_split"]:
                                    c1 = c0 + nchk
                                    _eng(nc, qn).dma_start(
                                        out=in_b[:, c0 * C : c1 * C],
                                        in_=inp[b, :, c0:c1, :],
                                    )
                                    c0 = c1
                                assert c0 == NCH
                            else:
                                q_in[b % len(q_in)].dma_start(
                                    out=in_b[:, :],
                                    in_=inp[b].rearrange("p j c -> p (j c)"),
                                )
                    # batch-wide output accumulator (both channel halves)
                    o_t = o_pool.tile(
                        [128, 2 * L], mybir.dt.bfloat16, tag="o", name="o_t"
                    )

                    # ---- matmuls -> psum -> compact bf16 copies ----
                    for r in range(n_groups // g_per_ps):  # 8 psum tiles
                        pss = [
                            ps_pool.tile(
                                [128, 512],
                                mybir.dt.float32,
                                tag=f"ps{ch}",
                                name=f"ps{ch}",
                            )
                            for ch in range(2)
                        ]
                        for j in range(4):      # 4 chunks per psum tile
                            gp = r * 4 + j
                            for ch in range(2):
                                ih = in_b[
                                    :,
                                    gp * C + ch * 128 : gp * C + ch * 128 + 128,
                                ]
                                bh = bd_r[:, 0:NPH, e, gp * M : (gp + 1) * M]
                                if not cfg["abl_no_main"]:
                                    nc.tensor.matmul(
                                        pss[ch][:, j * 128 : (j + 1) * 128],
                                        ih,
                                        bh,
                                        start=True,
                                        stop=cfg["abl_no_pair"],
                                        skip_group_check=True,
                                    )
                                # K=128 like the mains (uniform PE tile
                                # config — mixed sizes stall ~150ns/switch,
                                # measured ~20us/core): region-4 rows
                                # [10,128) are persistent zeros, so the
                                # full-chunk contraction adds nothing.
                                ihb = in_b[
                                    :,
                                    (gp + 1) * C
                                    + ch * 128 : (gp + 1) * C
                                    + ch * 128
                                    + 128,
                                ]
                                bhb = bd_r[
                                    :, nreg - 1, e, gp * M : (gp + 1) * M
                                ]
                                if not cfg["abl_no_pair"]:
                                    nc.tensor.matmul(
                                        pss[ch][:, j * 128 + 128 - M : (j + 1) * 128],
                                        ihb,
                                        bhb,
                                        start=cfg["abl_no_main"],
                                        stop=True,
                                        skip_group_check=True,
                                    )
                        for ch in range(2):
                            if cfg["abl_no_copy"]:
                                continue
                            _copy(
                                e_cp[ch],
                                o_t[
                                    :,
                                    ch * L + r * 512 : ch * L + (r + 1) * 512,
                                ],
                                pss[ch][:, :],
                            )

                    # ---- store(s) per batch ----
                    if not cfg["abl_no_store"]:
                        if cfg["store_split"]:
                            nck = cfg["store_chunks"]
                            for ch in range(2):
                                for k in range(nck):
                                    lo = k * L // nck
                                    hi = (k + 1) * L // nck
                                    q_sts[(b * 2 + ch) % len(q_sts)].dma_start(
                                        out=outT[
                                            b, ch * 128 : (ch + 1) * 128, lo:hi
                                        ],
                                        in_=o_t[:, ch * L + lo : ch * L + hi],
                                    )
                        else:
                            q_sts[b % len(q_sts)].dma_start(
                                out=outT[b].rearrange("(h p) w -> p h w", h=2),
                                in_=o_t.rearrange("p (h w) -> p h w", h=2),
                            )
                if tick is not None:
                    # flush the HWDGE queues: same-queue reads complete only
                    # after all prior writes on that queue.  (Hoisting this
                    # out of the repeat loop measured neutral-to-worse —
                    # the tiny flush DMAs overlap the next repeat's loads.)
                    fl = o_pool.tile([2, C], mybir.dt.bfloat16, tag="fl")
                    nc.sync.dma_start(out=fl[0:1, :], in_=outT[0, 0:1, 0:C])
                    nc.scalar.dma_start(out=fl[1:2, :], in_=outT[0, 128:129, 0:C])
                    nc.sync.dma_start(out=tick[:, :], in_=fl[0:1, :])
                    nc.sync.dma_start(out=tick[:, 0:C], in_=fl[1:2, :])
    nc.compile()
    return nc


BF16 = ml_dtypes.bfloat16


def _prep_core(x: np.ndarray, w: np.ndarray, M: int):
    """x: [B_LOC, L, C] f32, w: [B_LOC, L, T] f32 -> (in_chunks, band), bf16.

    in_chunks: [B_LOC, 128, NCH, C] partition-major (row 128*q + p of the
    zero-padded input at [b, p, q, :]).
    band: [NPAIR, 128, NREG, 2, GCH*M]; for group (gp, f) of batch b, col
    gp*M + m', the value w[b, 128*gp + M*f + m', tau] sits at row
    p = M*f + m' + tau — in region f while p < 128, wrapped to row p-128 of
    the last region otherwise; batch pairs interleave on the 4th axis.
    """
    NPH = 128 // M
    nreg = NREG[M]
    nb = x.shape[0]
    in_pad = np.zeros((nb, LPAD, C), BF16)
    in_pad[:, D : D + L, :] = x.astype(BF16)
    in_chunks = np.ascontiguousarray(
        in_pad.reshape(nb, NCH, 128, C).transpose(0, 2, 1, 3)
    )
    w16 = w.astype(BF16)
    bandm = np.zeros((nb, 128, nreg, GCH * M), BF16)
    mp = np.arange(M)
    gp = np.arange(GCH)
    cols = gp[:, None] * M + mp[None, :]            # [GCH, M]
    for f in range(NPH):
        t = 128 * gp[:, None] + M * f + mp[None, :]  # [GCH, M]
        for tau in range(T):
            p = M * f + mp + tau                     # [M]
            rg = np.where(p < 128, f, nreg - 1)
            pw = p % 128
            rgb = np.broadcast_to(rg[None, :], (GCH, M))
            pwb = np.broadcast_to(pw[None, :], (GCH, M))
            bandm[:, pwb, rgb, cols] = w16[:, t, tau]
    band = np.ascontiguousarray(
        bandm.reshape(nb // 2, 2, 128, nreg, GCH * M).transpose(0, 2, 3, 1, 4)
    )
    return in_chunks, band


def kernel(inputs: np.ndarray, weights: np.ndarray) -> np.ndarray:
    global LAST_RESULT
    inputs = np.ascontiguousarray(np.asarray(inputs, dtype=np.float32))
    weights = np.ascontiguousarray(np.asarray(weights, dtype=np.float32))
    assert inputs.shape == (B, L, C) and weights.shape == (B, L, T)

    if "nc" not in _CACHE:
        _CACHE["nc"] = _build_nc()
    nc = _CACHE["nc"]

    in_maps = []
    for c in range(N_CORES):
        sl = slice(c * B_LOC, (c + 1) * B_LOC)
        ip, bd = _prep_core(inputs[sl], weights[sl], CFG["group_m"])
        in_maps.append({"in_chunks": ip, "band": bd})

    res = run_bass_kernel_spmd(nc, in_maps, core_ids=list(range(N_CORES)))
    LAST_RESULT = res
    # outputs come back channel-major bf16 [B_LOC, C, L]; un-transpose and
    # widen to f32 on host
    return np.ascontiguousarray(
        np.concatenate(
            [
                r["outT"].astype(np.float32).transpose(0, 2, 1)
                for r in res.results
            ],
            axis=0,
        )
    )
